# revision 25
# baseline (speedup 1.0000x reference)
"""Deformable Conv2d (DCNv2-style) Trainium2 Bass kernel — input-space gather.

Shards over 8 NeuronCores: core = b * 2 + ph  (b = batch 0..3, ph = pixel half).
Each core computes output pixels [ph*2048, (ph+1)*2048) of batch b.

Device pipeline per core:
  1. offset/mask 3x3 conv emitted PIXEL-MAJOR directly (PE, free=27/18 per
     tile; bias via a 1-row ones matmul) into cP (A-order) and cPB (B-order).
  2. bilinear coords/coeffs on DVE (floor via RNE int cast of v-0.5); the
     B-ordered pipeline computes gather row indices so the dma_gather idx
     wrap ([16, n/16] partition layout) is a plain PE transpose.
  3. x^T (pixel-major [pix, C] f16) staged to DRAM via PE transposes.
  4. dma_gather of x^T row-pairs (512B descriptors) at data-dependent rows:
     each descriptor = corners (y, x0), (y, x0+1) for one (pixel, tap).
  5. corner combine via diag-scaled transpose-matmuls: for each corner,
     matmul(lhsT=gathered [pix,C] tile, rhs=diag(beta)) accumulates
     s_k[C, pix] in PSUM; diag(beta) built by one tensor_scalar on the
     identity (beta rides the partition dim).
  6. main conv: out[O, pix] += W_k[C,O]^T @ s_k[C, pix] over 9 taps (PE).
  7. store [O, 2048] f32; host reassembles + bias.
"""
import sys

sys.path.insert(0, "/opt/trn_rl_repo")

import numpy as np

import concourse.mybir as mybir
from concourse.ap import AP
from concourse.bacc import Bacc
from concourse.tile import TileContext
from concourse import bass_utils

F32 = mybir.dt.float32
F16 = mybir.dt.float16
I32 = mybir.dt.int32
I16 = mybir.dt.int16
Alu = mybir.AluOpType
Act = mybir.ActivationFunctionType

B, C, H, W = 4, 128, 64, 64
O, K, KK = 128, 3, 9
HWp = H * W
HALF = HWp // 2              # 2048 pixels per core
HROWS = 32
XR = 38                      # local halo rows: global [h0-3, h0+35); |dy|<2 safe
XPIX = XR * W                # 2432 -> 2560 padded; 20 tiles of 128
NT = XPIX // 128             # 20 x tiles
UR = XPIX + 2                # x^T staging rows (zero rows 0 and UR-1)
G16 = HALF // 128            # 16 pixel groups


def _colsA(tile, row0, nrow=9):
    """A-pipeline view: [128, 16g x nrow] cols of cP (stride 27)."""
    t = tile[:, :]
    return AP(tensor=t.tensor, offset=t.offset + row0,
              ap=[t.ap[0], [27, G16], [1, nrow]])


def _colsB(tile, row0, nrow=9):
    t = tile[:, :]
    return AP(tensor=t.tensor, offset=t.offset + row0,
              ap=[t.ap[0], [18, G16], [1, nrow]])


def _build(nc, debug=False):
    xh = nc.dram_tensor("xh", [C, XPIX], F16, kind="ExternalInput")
    wc = nc.dram_tensor("wc", [C, 9 * 27], F16, kind="ExternalInput")
    mbias = nc.dram_tensor("mbias", [128, 144], F32, kind="ExternalInput")
    wkt = nc.dram_tensor("wkt", [C, KK * O], F16, kind="ExternalInput")
    gyA = nc.dram_tensor("gyA", [128, 144], F32, kind="ExternalInput")
    gxA = nc.dram_tensor("gxA", [128, 144], F32, kind="ExternalInput")
    gyB = nc.dram_tensor("gyB", [128, 144], F32, kind="ExternalInput")
    gxB = nc.dram_tensor("gxB", [128, 144], F32, kind="ExternalInput")
    shiftv = nc.dram_tensor("shiftv", [128, 1], F32, kind="ExternalInput")
    ident = nc.dram_tensor("ident", [128, 128], F16, kind="ExternalInput")
    identf = nc.dram_tensor("identf", [128, 128], F32, kind="ExternalInput")
    out = nc.dram_tensor("out", [O, HALF], F32, kind="ExternalOutput")

    xstage = nc.dram_tensor("xstage", [UR * 128], F16, kind="Internal")
    idxstage = nc.dram_tensor("idxstage", [16 * 18 * 128], I16, kind="Internal")
    if debug:
        dbg_conv = nc.dram_tensor("dbg_conv", [27, HALF], F32, kind="ExternalOutput")
        dbg_cT = nc.dram_tensor("dbg_cT", [128, G16 * 36], F32, kind="ExternalOutput")
        dbg_xT = nc.dram_tensor("dbg_xT", [UR, 128], F16, kind="ExternalOutput")
        dbg_gt = nc.dram_tensor("dbg_gt", [128, 32 * 256], F16, kind="ExternalOutput")
        dbg_s16 = nc.dram_tensor("dbg_s16", [128, 128], F16, kind="ExternalOutput")
        dbg_idx = nc.dram_tensor("dbg_idx", [128, 18 * 128], I16, kind="ExternalOutput")

    with TileContext(nc) as tc:
        with (
            tc.tile_pool(name="big", bufs=1) as big,
            tc.tile_pool(name="small", bufs=1) as small,
        ):
            x_sb = big.tile([C, XPIX], F16, tag="x_sb")
            nc.gpsimd.dma_start(x_sb[:, :], xh[:, :])
            wc_sb = small.tile([C, 9 * 27], F16, tag="wc")
            nc.scalar.dma_start(wc_sb[:, :], wc[:, :])
            mb_sb = small.tile([128, 144], F32, tag="mbias")
            nc.scalar.dma_start(mb_sb[:, :], mbias[:, :])
            wk_sb = big.tile([C, KK * O], F16, tag="wk")
            nc.sync.dma_start(wk_sb[:, :], wkt[:, :])
            gyA_sb = small.tile([128, 144], F32, tag="gyA")
            nc.scalar.dma_start(gyA_sb[:, :], gyA[:, :])
            gxA_sb = small.tile([128, 144], F32, tag="gxA")
            nc.scalar.dma_start(gxA_sb[:, :], gxA[:, :])
            gyB_sb = small.tile([128, 144], F32, tag="gyB")
            nc.scalar.dma_start(gyB_sb[:, :], gyB[:, :])
            gxB_sb = small.tile([128, 144], F32, tag="gxB")
            nc.scalar.dma_start(gxB_sb[:, :], gxB[:, :])
            shift_sb = small.tile([128, 1], F32, tag="shiftv")
            nc.scalar.dma_start(shift_sb[:, :], shiftv[:, :])
            id_sb = small.tile([128, 128], F16, tag="ident")
            nc.scalar.dma_start(id_sb[:, :], ident[:, :])
            idf_sb = small.tile([128, 128], F32, tag="identf")
            nc.scalar.dma_start(idf_sb[:, :], identf[:, :])

            # padded conv input: local rows 2..36 -> [C, 34*66], zero borders
            xpad = big.tile([C, 34 * 66], F16, tag="xpad")
            nc.gpsimd.memset(xpad[:, :], 0.0)
            nc.vector.tensor_copy(
                AP(tensor=xpad.tensor, offset=xpad[:, :].offset + 1,
                   ap=[xpad[:, :].ap[0], [66, 34], [1, W]]),
                AP(tensor=x_sb.tensor, offset=x_sb[:, :].offset + 2 * W,
                   ap=[x_sb[:, :].ap[0], [W, 34], [1, W]]),
            )

            # ---------- x^T staging to DRAM (pixel-major [UR, C] f16) -------
            zrow = small.tile([1, 128], F16, tag="zrow")
            nc.vector.memset(zrow[:, :], 0.0)
            for uoff in (0, (UR - 1) * 128):
                nc.scalar.dma_start(
                    AP(tensor=xstage, offset=uoff, ap=[[1, 128]]),
                    zrow[0:1, :])
            with (
                tc.tile_pool(name="ptx", bufs=3, space="PSUM") as ptx,
                tc.tile_pool(name="xtp", bufs=3) as xtp,
            ):
                for t in range(NT):
                    pt = ptx.tile([128, 128], F16, tag="ptx")
                    nc.tensor.transpose(
                        pt[:, :], x_sb[:, t * 128:(t + 1) * 128], id_sb[:, :])
                    xt16 = xtp.tile([128, 128], F16, tag="xt16")
                    nc.vector.tensor_copy(xt16[:, :], pt[:, :])
                    nc.sync.dma_start(
                        AP(tensor=xstage, offset=(1 + t * 128) * 128,
                           ap=[[128, 128], [1, 128]]),
                        xt16[:, :])

            # ---------- offset/mask conv (channel-major) + pixel-major copies
            convR = big.tile([27, HALF], F32, tag="convR")
            cP = big.tile([128, G16 * 27], F32, tag="cP")
            cPB = big.tile([128, G16 * 18], F32, tag="cPB")
            xp0 = xpad[:, :]
            with tc.tile_pool(name="pcv", bufs=2, space="PSUM") as pcv:
                for ch in range(4):
                    pc = pcv.tile([27, 512], F32, tag="pc")
                    for th in range(3):
                        for tw in range(3):
                            tap = th * 3 + tw
                            rhs = AP(
                                tensor=xpad.tensor,
                                offset=xp0.offset + (ch * 8 + th) * 66 + tw,
                                ap=[xp0.ap[0], [66, 8], [1, W]])
                            nc.tensor.matmul(
                                pc[:, :], wc_sb[:, tap * 27:(tap + 1) * 27],
                                rhs, start=(tap == 0), stop=(tap == 8))
                    if ch % 2 == 0:
                        nc.vector.tensor_copy(
                            convR[0:27, ch * 512:(ch + 1) * 512], pc[:, :])
                    else:
                        nc.scalar.activation(
                            convR[0:27, ch * 512:(ch + 1) * 512], pc[:, :],
                            Act.Copy)
                # B-order conv copy: col P*16+g -> convB[:, g*128+P]
                convB = big.tile([18, HALF], F32, tag="convB")
                cB = convB[:, :]
                nc.vector.tensor_copy(
                    AP(tensor=cB.tensor, offset=cB.offset,
                       ap=[cB.ap[0], [1, HALF]]),
                    AP(tensor=convR.tensor, offset=convR[:, :].offset,
                       ap=[[convR[:, :].ap[0][0], 18], [1, G16], [16, 128]]),
                )
                for g in range(G16):
                    ptb = pcv.tile([128, 18], F32, tag="ptb")
                    nc.tensor.transpose(
                        ptb[:, :], convB[:, g * 128:(g + 1) * 128],
                        idf_sb[0:18, 0:18])
                    nc.vector.tensor_copy(cPB[:, g * 18:(g + 1) * 18], ptb[:, :])
                for g in range(G16):
                    pta = pcv.tile([128, 27], F32, tag="pta")
                    nc.tensor.transpose(
                        pta[:, :], convR[:, g * 128:(g + 1) * 128],
                        idf_sb[0:27, 0:27])
                    if g % 2 == 0:
                        nc.vector.tensor_copy(cP[:, g * 27:(g + 1) * 27], pta[:, :])
                    else:
                        nc.scalar.activation(cP[:, g * 27:(g + 1) * 27],
                                             pta[:, :], Act.Copy)

            # ---------- A pipeline: coefficients (pixel-major, A-order) -----
            NSL = 20
            cw = big.tile([128, NSL * 144], F32, tag="cw")
            itmp = small.tile([128, 144], I32, tag="itmp")
            cT = big.tile([128, G16 * 36], F32, tag="cT")

            def S(q):
                return cw[:, q * 144:(q + 1) * 144]

            def emit_A():
                PY, PX, M, Y0, X0, FY, FX, Y1, X1 = range(9)
                CY0, CY1, VY0, VY1, VX0, VX1, IXC, T1, T2, T3 = range(9, 19)
                nc.vector.tensor_tensor(S(PY), _colsA(cP, 0), gyA_sb[:, :], Alu.add)
                nc.vector.tensor_tensor(S(PX), _colsA(cP, 9), gxA_sb[:, :], Alu.add)
                nc.vector.tensor_tensor(S(T1), _colsA(cP, 18), mb_sb[:, :], Alu.add)
                nc.scalar.activation(S(M), S(T1), Act.Sigmoid)
                # floors (RNE cast of v-0.5)
                nc.vector.tensor_scalar(S(T1), S(PY), -0.5, None, Alu.add)
                nc.vector.tensor_copy(itmp[:, :], S(T1))
                nc.vector.tensor_copy(S(Y0), itmp[:, :])
                nc.vector.tensor_scalar(S(T1), S(PX), -0.5, None, Alu.add)
                nc.vector.tensor_copy(itmp[:, :], S(T1))
                nc.vector.tensor_copy(S(X0), itmp[:, :])
                nc.vector.tensor_tensor(S(FY), S(PY), S(Y0), Alu.subtract)
                nc.vector.tensor_tensor(S(FX), S(PX), S(X0), Alu.subtract)
                nc.vector.tensor_scalar(S(Y1), S(Y0), 1.0, None, Alu.add)
                nc.vector.tensor_scalar(S(X1), S(X0), 1.0, None, Alu.add)
                # validity
                nc.vector.tensor_scalar(S(CY0), S(Y0), 0.0, 63.0, Alu.max, Alu.min)
                nc.vector.tensor_tensor(S(VY0), S(CY0), S(Y0), Alu.is_equal)
                nc.vector.tensor_scalar(S(CY1), S(Y1), 0.0, 63.0, Alu.max, Alu.min)
                nc.vector.tensor_tensor(S(VY1), S(CY1), S(Y1), Alu.is_equal)
                nc.vector.tensor_scalar(S(T1), S(X0), 0.0, 63.0, Alu.max, Alu.min)
                nc.vector.tensor_tensor(S(VX0), S(T1), S(X0), Alu.is_equal)
                nc.vector.tensor_scalar(S(T1), S(X1), 0.0, 63.0, Alu.max, Alu.min)
                nc.vector.tensor_tensor(S(VX1), S(T1), S(X1), Alu.is_equal)
                # weights: wy0=(1-fy)*m*vy0 ; wy1=fy*m*vy1 ; ax0=(1-fx)*vx0 ; ax1=fx*vx1
                nc.vector.tensor_scalar(S(T1), S(FY), -1.0, 1.0, Alu.mult, Alu.add)
                nc.vector.tensor_tensor(S(T1), S(T1), S(M), Alu.mult)
                nc.vector.tensor_tensor(S(T1), S(T1), S(VY0), Alu.mult)     # wy0
                nc.vector.tensor_tensor(S(T2), S(FY), S(M), Alu.mult)
                nc.vector.tensor_tensor(S(T2), S(T2), S(VY1), Alu.mult)     # wy1
                nc.vector.tensor_scalar(S(T3), S(FX), -1.0, 1.0, Alu.mult, Alu.add)
                nc.vector.tensor_tensor(S(T3), S(T3), S(VX0), Alu.mult)     # ax0
                nc.vector.tensor_tensor(S(FX), S(FX), S(VX1), Alu.mult)     # ax1

                def cT_view(corner):
                    t = cT[:, :]
                    return AP(tensor=t.tensor, offset=t.offset + corner * 9,
                              ap=[t.ap[0], [36, G16], [1, 9]])

                nc.vector.tensor_tensor(cT_view(0), S(T1), S(T3), Alu.mult)  # c00
                nc.vector.tensor_tensor(cT_view(1), S(T1), S(FX), Alu.mult)  # c01
                nc.vector.tensor_tensor(cT_view(2), S(T2), S(T3), Alu.mult)  # c10
                nc.vector.tensor_tensor(cT_view(3), S(T2), S(FX), Alu.mult)  # c11

            # ---------- B pipeline: gather indices (slot P*16+g order) ------
            bw = big.tile([128, 8 * 144], F32, tag="bw")
            idxPM = big.tile([128, 288], F32, tag="idxPM")

            def Sb(q):
                return bw[:, q * 144:(q + 1) * 144]

            BPY, BPX, BY0, BX0, BT, BIX, BCY, BT2 = range(8)
            nc.vector.tensor_tensor(Sb(BPY), _colsB(cPB, 0), gyB_sb[:, :], Alu.add)
            nc.vector.tensor_tensor(Sb(BPX), _colsB(cPB, 9), gxB_sb[:, :], Alu.add)
            nc.vector.tensor_scalar(Sb(BT), Sb(BPY), -0.5, None, Alu.add)
            nc.vector.tensor_copy(itmp[:, :], Sb(BT))
            nc.vector.tensor_copy(Sb(BY0), itmp[:, :])
            nc.vector.tensor_scalar(Sb(BT), Sb(BPX), -0.5, None, Alu.add)
            nc.vector.tensor_copy(itmp[:, :], Sb(BT))
            nc.vector.tensor_copy(Sb(BX0), itmp[:, :])
            nc.vector.tensor_scalar(Sb(BIX), Sb(BX0), -1.0, 63.0, Alu.max, Alu.min)

            def idx_view(pair):
                t = idxPM[:, :]
                return AP(tensor=t.tensor, offset=t.offset + pair * 144,
                          ap=[t.ap[0], [1, G16], [16, 9]])

            # idx0 = clamp(y0)*64 + shift + ix
            nc.vector.tensor_scalar(Sb(BCY), Sb(BY0), 0.0, 63.0, Alu.max, Alu.min)
            nc.vector.tensor_scalar(Sb(BT2), Sb(BCY), 64.0, shift_sb[:, 0:1],
                                    Alu.mult, Alu.add)
            nc.vector.tensor_tensor(Sb(BT2), Sb(BT2), Sb(BIX), Alu.add)
            nc.vector.tensor_scalar(idx_view(0), Sb(BT2), 0.0, float(UR - 2),
                                    Alu.max, Alu.min)
            # idx1 = clamp(y0+1)*64 + shift + ix
            nc.vector.tensor_scalar(Sb(BCY), Sb(BY0), 1.0, None, Alu.add)
            nc.vector.tensor_scalar(Sb(BCY), Sb(BCY), 0.0, 63.0, Alu.max, Alu.min)
            nc.vector.tensor_scalar(Sb(BT2), Sb(BCY), 64.0, shift_sb[:, 0:1],
                                    Alu.mult, Alu.add)
            nc.vector.tensor_tensor(Sb(BT2), Sb(BT2), Sb(BIX), Alu.add)
            nc.vector.tensor_scalar(idx_view(1), Sb(BT2), 0.0, float(UR - 2),
                                    Alu.max, Alu.min)

            # idx transposes -> wrap rows [16, 128] each, cast to i16
            wrapS = big.tile([16, 18 * 128], I16, tag="wrapS")
            with tc.tile_pool(name="psi", bufs=4, space="PSUM") as psi:
                for pair in range(2):
                    for k in range(KK):
                        pw = psi.tile([16, 128], F32, tag="pw")
                        nc.tensor.transpose(
                            pw[:, :],
                            idxPM[:, pair * 144 + k * 16: pair * 144 + (k + 1) * 16],
                            idf_sb[:, :])
                        r = k * 2 + pair
                        nc.vector.tensor_copy(
                            wrapS[:, r * 128:(r + 1) * 128], pw[:, :])
            # bounce to DRAM and back replicated x8
            nc.scalar.dma_start(
                AP(tensor=idxstage, offset=0, ap=[[2304, 16], [1, 2304]]),
                wrapS[:, :])
            idxW = big.tile([128, 18 * 128], I16, tag="idxW")
            for a in range(2):
                nc.scalar.dma_start(
                    idxW[a * 64:(a + 1) * 64, :],
                    AP(tensor=idxstage, offset=0,
                       ap=[[0, 4], [2304, 16], [1, 2304]]))

            emit_A()
            if debug:
                nc.sync.dma_start(dbg_conv[:, :], convR[:, :])
                nc.sync.dma_start(dbg_cT[:, :], cT[:, :])
                nc.sync.dma_start(
                    AP(tensor=dbg_xT, offset=0, ap=[[128, UR], [1, 128]]),
                    AP(tensor=xstage, offset=0, ap=[[128, UR], [1, 128]]))
                nc.sync.dma_start(dbg_idx[:, :], idxW[:, :])

            # ---------- gathers + combine + main conv ----------
            def emit_diag(which, dst, col):
                if which == 0:
                    nc.vector.tensor_scalar(dst, id_sb[:, :], col, None, Alu.mult)
                elif which == 1:
                    nc.scalar.activation(dst, id_sb[:, :], Act.Copy, scale=col)
                else:
                    nc.gpsimd.tensor_scalar(dst, id_sb[:, :], col, None, Alu.mult)

            def emit_copy(which, dst, src):
                if which == 0:
                    nc.vector.tensor_copy(dst, src)
                elif which == 1:
                    nc.scalar.activation(dst, src, Act.Copy)
                else:
                    nc.gpsimd.tensor_copy(dst, src)

            # weighted engine schedule: DVE fast, ACT/Pool slower
            SCHED = [0, 0, 1, 0, 2, 0, 1, 0, 2]

            with (
                tc.tile_pool(name="gat", bufs=3) as gat,
                tc.tile_pool(name="dg", bufs=80) as dgp,
                tc.tile_pool(name="s16p", bufs=3) as s16p,
                tc.tile_pool(name="pss", bufs=4, space="PSUM") as pss,
                tc.tile_pool(name="pso", bufs=1, space="PSUM") as pso,
                tc.tile_pool(name="osb", bufs=1) as osbp,
            ):
                po = pso.tile([128, HALF], F32, tag="po")
                src_ap = AP(tensor=xstage, offset=0,
                            ap=[[128, UR - 1], [1, 256]])
                ei = 0
                for k in range(KK):
                    gt = gat.tile([128, 2 * G16, 256], F16, tag="gt")
                    nc.gpsimd.dma_gather(
                        gt[:, :, :], src_ap,
                        idxW[:, k * 256:(k + 1) * 256],
                        2 * HALF, 2 * HALF, 256, elem_step=128,
                        single_packet=False)
                    # build all 64 diags for this k ahead of the matmuls so
                    # DVE/ACT/Pool run ahead of PE
                    dgs = []
                    for g in range(G16):
                        for j in range(4):
                            dg = dgp.tile([128, 128], F16, tag="dg")
                            emit_diag(
                                SCHED[ei % len(SCHED)], dg[:, :],
                                cT[:, g * 36 + j * 9 + k:
                                   g * 36 + j * 9 + k + 1])
                            ei += 1
                            dgs.append(dg)
                    s16q = None
                    for g in range(G16):
                        sp = pss.tile([128, 128], F32, tag="sp")
                        for j in range(4):
                            pair, half = j // 2, j % 2
                            nc.tensor.matmul(
                                sp[:, :],
                                gt[:, pair * G16 + g,
                                   half * 128:(half + 1) * 128],
                                dgs[g * 4 + j][:, :],
                                start=(j == 0), stop=(j == 3))
                        if g % 4 == 0:
                            s16q = s16p.tile([128, 512], F16, tag="s16")
                        # gpsimd cannot read PSUM; alternate DVE/ACT only
                        emit_copy(g % 2, s16q[:, (g % 4) * 128:
                                               (g % 4 + 1) * 128], sp[:, :])
                        if debug and k == 0 and g == 0:
                            nc.sync.dma_start(
                                dbg_gt[:, :],
                                gt[:, :, :].rearrange("p a b -> p (a b)"))
                            nc.sync.dma_start(dbg_s16[:, :],
                                              s16q[:, 0:128])
                        if g % 4 == 3:
                            # one 512-wide (= one full PSUM bank) accumulation
                            # group per bank: start/stop exactly once per bank
                            nc.tensor.matmul(
                                po[:, (g // 4) * 512:(g // 4 + 1) * 512],
                                wk_sb[:, k * O:(k + 1) * O], s16q[:, :],
                                start=(k == 0), stop=(k == KK - 1))
                osb = osbp.tile([128, HALF], F32, tag="osb")
                for q in range(4):
                    sl = slice(q * 512, (q + 1) * 512)
                    nc.vector.tensor_copy(osb[:, sl], po[:, sl])
                nc.sync.dma_start(out[:, :], osb[:, :])

    nc.compile()
    return nc


_CACHE = {}


def _get_nc(debug=False):
    if debug not in _CACHE:
        nc = Bacc()
        _CACHE[debug] = _build(nc, debug=debug)
    return _CACHE[debug]


def _grid_tables(h0, order):
    """[128, 144] tables: [P, g*9+k] = gy/gx of (pixel, k) for the given
    slot->pixel order: 'A': pixel = g*128+P ; 'B': pixel = P*16+g."""
    ki = (np.arange(KK) // 3).astype(np.float32)
    kj = (np.arange(KK) % 3).astype(np.float32)
    P = np.arange(128)
    g = np.arange(G16)
    if order == "A":
        pix = g[None, :] * 128 + P[:, None]          # [128, 16]
    else:
        pix = P[:, None] * 16 + g[None, :]
    gy = (h0 + pix // W)[:, :, None] + (ki - 1.0)[None, None, :]
    gx = (pix % W)[:, :, None] + (kj - 1.0)[None, None, :]
    return (np.ascontiguousarray(gy.reshape(128, 144).astype(np.float32)),
            np.ascontiguousarray(gx.reshape(128, 144).astype(np.float32)))


def _prep_inputs(x, w_off, b_off, w_mask, b_mask, weight, bias):
    x = np.asarray(x, np.float32)
    w_off = np.asarray(w_off, np.float32)
    b_off = np.asarray(b_off, np.float32)
    w_mask = np.asarray(w_mask, np.float32)
    b_mask = np.asarray(b_mask, np.float32)
    weight = np.asarray(weight, np.float32)

    w_cat = np.concatenate([w_off[0::2], w_off[1::2], w_mask], axis=0)
    b_cat = np.concatenate([b_off[0::2], b_off[1::2], b_mask])
    wc = np.ascontiguousarray(
        w_cat.reshape(27, C, 9).transpose(1, 2, 0).reshape(C, 9 * 27)).astype(np.float16)
    # conv biases: y/x folded into the grid tables, mask as an add-table
    by = b_cat[0:9].astype(np.float32)       # y-offset bias per k
    bx = b_cat[9:18].astype(np.float32)
    bm = b_cat[18:27].astype(np.float32)
    mbias = np.ascontiguousarray(
        np.tile(bm[None, None, :], (128, G16, 1)).reshape(128, 144))
    wkt = np.ascontiguousarray(
        weight.reshape(O, C, KK).transpose(1, 2, 0).reshape(C, KK * O)).astype(np.float16)
    ident = np.eye(128, dtype=np.float16)
    identf = np.eye(128, dtype=np.float32)

    in_maps = []
    for core in range(8):
        b = core // 2
        ph = core % 2
        h0 = ph * HROWS
        hl = h0 - 3
        xb = x[b].reshape(C, H, W)
        xhh = np.zeros((C, XR, W), np.float32)
        for r in range(XR):
            gr = hl + r
            if 0 <= gr < H:
                xhh[:, r] = xb[:, gr]
        gyA, gxA = _grid_tables(h0, "A")
        gyB, gxB = _grid_tables(h0, "B")
        badd = np.tile(by[None, None, :], (128, G16, 1)).reshape(128, 144)
        bxadd = np.tile(bx[None, None, :], (128, G16, 1)).reshape(128, 144)
        gyA = np.ascontiguousarray(gyA + badd)
        gxA = np.ascontiguousarray(gxA + bxadd)
        gyB = np.ascontiguousarray(gyB + badd)
        gxB = np.ascontiguousarray(gxB + bxadd)
        shiftv = np.full((128, 1), 1.0 - hl * 64.0, np.float32)
        in_maps.append({
            "xh": np.ascontiguousarray(xhh.reshape(C, XPIX)).astype(np.float16),
            "wc": wc, "mbias": mbias, "wkt": wkt,
            "gyA": gyA, "gxA": gxA, "gyB": gyB, "gxB": gxB,
            "shiftv": shiftv, "ident": ident, "identf": identf,
        })
    return in_maps


def kernel(x, w_off, b_off, w_mask, b_mask, weight, bias, _trace=False,
           _debug=False):
    nc = _get_nc(debug=_debug)
    in_maps = _prep_inputs(x, w_off, b_off, w_mask, b_mask, weight, bias)
    res = bass_utils.run_bass_kernel_spmd(
        nc, in_maps, core_ids=list(range(8)), trace=_trace)
    out = np.zeros((B, O, H, W), np.float32)
    for core in range(8):
        b, ph = core // 2, core % 2
        chunk = res.results[core]["out"]           # [O, HALF]
        out[b, :, ph * HROWS:(ph + 1) * HROWS, :] = (
            chunk.reshape(O, HROWS, W))
    out += np.asarray(bias, np.float32)[None, :, None, None]
    if _trace or _debug:
        kernel._last = res
    return out
